# revision 1
# baseline (speedup 1.0000x reference)
"""Trainium2 Bass kernel for an attention-augmented LSTM (CaptioningRNN).

Reference computation (per batch n, T timesteps):
    A_flat = A.reshape(N, H, 16); h0 = c0 = A_flat.mean(-1)
    scores_t = (h_{t-1} @ A_flat) / sqrt(H); w = softmax(scores)
    attn_t = A_flat @ w
    a = x_t @ Wx + h_{t-1} @ Wh + attn_t @ Wattn + b
    i, f, o, g = split(a, 4); c_t = sig(f)*c + sig(i)*tanh(g); h_t = sig(o)*tanh(c_t)

Strategy: data-parallel over batch across 8 cores (32 batch rows each).
Per core:
  Phase A: U = x @ Wx + b precomputed for all timesteps (bf16 weights,
           rows t-major) and staged to DRAM in bf16.
  Phase B: 64 recurrent steps. Gate matmul = [h; attn] (2048-dim contraction,
           bf16) against W2 = [Wh; Wattn] with gate-interleaved columns so each
           512-column block yields a full 128-dim slice of (i,f,o,g) and thus a
           128-dim slice of h/c. Attention scores are computed on the tensor
           engine (hT^T @ AT giving all batch pairs, diagonal extracted via a
           mask + strided reduce), softmax on ACT/DVE, attention pooling on DVE,
           h transposed back to hT layout via the DMA transpose xbar.

Weight-matrix column order (gate interleave): block j (512 cols) holds
original columns [i|f|o|g][j*128:(j+1)*128]. The same permutation is applied
to Wx, b and hence U.
"""

import math
import os

import numpy as np
import ml_dtypes

import concourse.bass as bass
import concourse.mybir as mybir
import concourse.tile as tile
from concourse import bacc

N, T, D, H = 256, 64, 1024, 1024
NCORES = 8
NB = N // NCORES          # 32 batch rows per core
G = 4 * H                 # 4096 gate columns
P = 16                    # attention positions (4x4)
KH = H // 128             # 8 contraction chunks for h
K2 = (2 * H) // 128       # 16 contraction chunks for [h; attn]
GB = G // 512             # 8 gate blocks of 512
F32 = mybir.dt.float32
BF16 = mybir.dt.bfloat16
BF = ml_dtypes.bfloat16

AF = mybir.ActivationFunctionType
ALU = mybir.AluOpType
AXX = mybir.AxisListType.X

_NC_CACHE = {}


def _gate_perm():
    """perm[new_col] = old_col for the gate-interleaved layout."""
    perm = np.empty(G, dtype=np.int64)
    for j in range(GB):
        for s in range(4):  # i, f, o, g
            perm[j * 512 + s * 128:(j * 512 + (s + 1) * 128)] = np.arange(
                s * H + j * 128, s * H + (j + 1) * 128)
    return perm


def build_nc(t_steps=T):
    """Build the SPMD Bass program (identical on all cores)."""
    nc = bacc.Bacc("TRN2", target_bir_lowering=False, debug=False,
                   num_devices=NCORES)

    xT_d = nc.dram_tensor("xT", [D, t_steps * NB], BF16, kind="ExternalInput")
    wx_d = nc.dram_tensor("wx", [D, G], BF16, kind="ExternalInput")
    w2_d = nc.dram_tensor("w2", [2 * H, G], BF16, kind="ExternalInput")
    b128_d = nc.dram_tensor("b128", [128, G], F32, kind="ExternalInput")
    at_d = nc.dram_tensor("at", [H, NB * P], BF16, kind="ExternalInput")
    h0T_d = nc.dram_tensor("h0T", [H, NB], BF16, kind="ExternalInput")
    h0q_d = nc.dram_tensor("h0q", [2 * 128, 128], F32, kind="ExternalInput")
    mask_d = nc.dram_tensor("mask", [NB, NB * P], BF16, kind="ExternalInput")
    ones_d = nc.dram_tensor("ones", [1, 128], BF16, kind="ExternalInput")
    ident_d = nc.dram_tensor("ident", [NB, NB], BF16, kind="ExternalInput")
    out_d = nc.dram_tensor("out", [NB, t_steps, H], F32, kind="ExternalOutput")

    n_row_tiles = (t_steps * NB) // 128

    with tile.TileContext(nc) as tc:
        with tc.tile_pool(name="dram", bufs=1, space="DRAM") as dpool:
            u_dram = dpool.tile([t_steps * NB, G], BF16)
            wdr = [dpool.tile([NB, P], BF16, tag=f"wdr{i}", name=f"wdr{i}")
                   for i in range(2)]

            # ---------------- Phase A: U = x @ Wx + b ----------------
            with tc.tile_pool(name="pa_res", bufs=1) as pa, \
                 tc.tile_pool(name="pa_ps", bufs=4, space="PSUM") as pa_ps, \
                 tc.tile_pool(name="pa_sb", bufs=6) as pa_sb:
                xT = []
                for d in range(KH):
                    t_ = pa.tile([128, t_steps * NB], BF16, tag=f"xT{d}")
                    nc.sync.dma_start(t_[:], xT_d[d * 128:(d + 1) * 128, :])
                    xT.append(t_)
                wx = []
                for d in range(KH):
                    t_ = pa.tile([128, G], BF16, tag=f"wx{d}")
                    nc.sync.dma_start(t_[:], wx_d[d * 128:(d + 1) * 128, :])
                    wx.append(t_)
                b128 = pa.tile([128, G], F32, tag="b128")
                nc.sync.dma_start(b128[:], b128_d[:])

                for m in range(n_row_tiles):
                    ms = slice(m * 128, (m + 1) * 128)
                    for g in range(GB):
                        gs = slice(g * 512, (g + 1) * 512)
                        ps = pa_ps.tile([128, 512], F32, tag="ps")
                        for d in range(KH):
                            nc.tensor.matmul(ps[:], xT[d][:, ms], wx[d][:, gs],
                                             start=(d == 0), stop=(d == KH - 1))
                        us = pa_sb.tile([128, 512], BF16, tag="us")
                        nc.vector.tensor_add(us[:], ps[:], b128[:, gs])
                        nc.sync.dma_start(u_dram[ms, gs], us[:])

            # ---------------- Phase B: recurrence ----------------
            with tc.tile_pool(name="res", bufs=1) as res, \
                 tc.tile_pool(name="ht", bufs=20) as htp, \
                 tc.tile_pool(name="u", bufs=2) as up, \
                 tc.tile_pool(name="st", bufs=2) as stp, \
                 tc.tile_pool(name="att", bufs=2) as attp, \
                 tc.tile_pool(name="abt", bufs=10) as abtp, \
                 tc.tile_pool(name="blk", bufs=2) as blkp, \
                 tc.tile_pool(name="psg", bufs=6, space="PSUM") as psg_p, \
                 tc.tile_pool(name="pss", bufs=1, space="PSUM") as pss_p, \
                 tc.tile_pool(name="psw", bufs=1, space="PSUM") as psw_p:

                w2 = []
                for k in range(K2):
                    t_ = res.tile([128, G], BF16, tag=f"w2_{k}")
                    nc.sync.dma_start(t_[:], w2_d[k * 128:(k + 1) * 128, :])
                    w2.append(t_)
                at_all = res.tile([128, KH * NB * P], BF16, tag="at_all")
                for k in range(KH):
                    nc.sync.dma_start(
                        at_all[:, k * NB * P:(k + 1) * NB * P],
                        at_d[k * 128:(k + 1) * 128, :])
                at = [at_all[:, k * NB * P:(k + 1) * NB * P]
                      for k in range(KH)]
                mask = res.tile([NB, NB * P], BF16, tag="mask")
                nc.sync.dma_start(mask[:], mask_d[:])
                ones = res.tile([1, 128], BF16, tag="ones")
                nc.sync.dma_start(ones[:], ones_d[:])
                ident = res.tile([NB, NB], BF16, tag="ident")
                nc.sync.dma_start(ident[:], ident_d[:])

                hT = []
                for k in range(KH):
                    t_ = htp.tile([128, NB], BF16, tag="ht")
                    nc.sync.dma_start(t_[:], h0T_d[k * 128:(k + 1) * 128, :])
                    hT.append(t_)
                c_b = []
                for q in range(2):
                    t_ = blkp.tile([128, 128], F32, tag="c", bufs=4,
                                   name=f"c0_{q}")
                    nc.sync.dma_start(t_[:], h0q_d[q * 128:(q + 1) * 128, :])
                    c_b.append(t_)

                u_t = up.tile([NB, G], BF16, tag="u")
                nc.sync.dma_start(u_t[:], u_dram[0:NB, :])

                inv_sqrt_h = 1.0 / math.sqrt(H)

                def smm(pg, k, lhs, rhs, start, stop):
                    if isinstance(lhs, tile.Tile):
                        lhs = lhs[:]
                    j = k % 2
                    nc.tensor.matmul(pg[64 * j:64 * j + NB, :], lhs, rhs,
                                     start=start, stop=stop,
                                     tile_position=(0, 64 * j),
                                     skip_group_check=True)

                def umm(pg, gsl, u):
                    nc.tensor.matmul(pg[0:NB, :], ident[:], u[:, gsl],
                                     start=False, stop=False,
                                     tile_position=(0, 0),
                                     skip_group_check=True)

                # ---- prologue: scores S_0 + h-part of blocks 0..3 + U ----
                ps_s = pss_p.tile([NB, NB * P], F32, tag="s")
                for k in range(KH):
                    nc.tensor.matmul(ps_s[:], hT[k][:], at[k],
                                     start=(k == 0), stop=(k == KH - 1))
                psg = {}
                for g in range(4):
                    gsl = slice(g * 512, (g + 1) * 512)
                    pg = psg_p.tile([128, 512], F32, tag="g", name=f"pg{g}")
                    for k in range(KH):
                        smm(pg, k, hT[k], w2[k][:, gsl], k < 2, False)
                    umm(pg, gsl, u_t)
                    psg[g] = pg

                for t in range(t_steps):
                    last = (t + 1 >= t_steps)
                    if not last:
                        u_next = up.tile([NB, G], BF16, tag="u")
                        nc.scalar.dma_start(
                            u_next[:], u_dram[(t + 1) * NB:(t + 2) * NB, :])

                    # (a) softmax chain for step t (scores psum -> w1)
                    sm_sc = nc.enter_named_scope(f"sm{t}", False)
                    masked = stp.tile([NB, NB * P], F32, tag="masked")
                    nc.vector.tensor_tensor(
                        out=masked[:].rearrange("m (p n) -> m p n", n=NB),
                        in0=ps_s[:].rearrange("m (n p) -> m p n", p=P),
                        in1=mask[:].rearrange("m (n p) -> m p n", p=P),
                        op=ALU.mult)
                    sc = stp.tile([NB, P], F32, tag="sc")
                    nc.vector.tensor_reduce(
                        sc[:], masked[:].rearrange("m (p n) -> m p n", n=NB),
                        axis=AXX, op=ALU.add)
                    # exp(x) = s/(1-s) with s = sigmoid(x): keeps the ACT
                    # table cache at {Sigmoid, Tanh} with no per-step reloads
                    sg = stp.tile([NB, P], F32, tag="sg")
                    nc.scalar.activation(sg[:], sc[:], AF.Sigmoid,
                                         scale=float(inv_sqrt_h))
                    om = stp.tile([NB, P], F32, tag="om")
                    nc.scalar.activation(om[:], sc[:], AF.Sigmoid,
                                         scale=float(-inv_sqrt_h))
                    omr = stp.tile([NB, P], F32, tag="omr")
                    nc.vector.reciprocal(omr[:], om[:])
                    expw = stp.tile([NB, P], F32, tag="expw")
                    nc.vector.tensor_tensor(out=expw[:], in0=sg[:], in1=omr[:],
                                            op=ALU.mult)
                    sume = stp.tile([NB, 1], F32, tag="sume")
                    nc.vector.tensor_reduce(sume[:], expw[:], axis=AXX,
                                            op=ALU.add)
                    rec = stp.tile([NB, 1], F32, tag="rec")
                    nc.vector.reciprocal(rec[:], sume[:])
                    w16 = stp.tile([NB, P], BF16, tag="w16")
                    nc.vector.tensor_scalar(out=w16[:], in0=expw[:],
                                            scalar1=rec[:], scalar2=None,
                                            op0=ALU.mult)
                    # flatten [NB, P] -> [1, NB*P]: direct SBUF->SBUF gather
                    w1 = stp.tile([1, NB * P], BF16, tag="w1")
                    nc.scalar.dma_start(w1[:], w16[:])
                    nc.leave_named_scope(f"sm{t}", sm_sc[0], False)

                    # (d4) block 4: h-part + U (covers the softmax latency)
                    sc_ = nc.enter_named_scope(f"d4_{t}", False)
                    for g in (4,):
                        gsl = slice(g * 512, (g + 1) * 512)
                        pg = psg_p.tile([128, 512], F32, tag="g",
                                        name=f"pg4_{g}")
                        for k in range(KH):
                            smm(pg, k, hT[k], w2[k][:, gsl], k < 2, False)
                        umm(pg, gsl, u_t)
                        psg[g] = pg
                    nc.leave_named_scope(f"d4_{t}", sc_[0], False)

                    # (b) broadcast w to 128 partitions via ones-matmul
                    sc_ = nc.enter_named_scope(f"att{t}", False)
                    ps_w = psw_p.tile([128, NB * P], F32, tag="w")
                    nc.tensor.matmul(ps_w[:], ones[:], w1[:],
                                     start=True, stop=True)
                    wfull = attp.tile([128, NB * P], BF16, tag="wfull")
                    nc.vector.tensor_copy(wfull[:], ps_w[:])

                    # (d5) block 5: h-part + U (covers the broadcast)
                    for g in (5,):
                        gsl = slice(g * 512, (g + 1) * 512)
                        pg = psg_p.tile([128, 512], F32, tag="g",
                                        name=f"pg5_{g}")
                        for k in range(KH):
                            smm(pg, k, hT[k], w2[k][:, gsl], k < 2, False)
                        umm(pg, gsl, u_t)
                        psg[g] = pg

                    # (e) attention pooling -> attnT: two wide DVE
                    # product+reduce pairs over the packed AT tile
                    attnT = []
                    with nc.allow_low_precision("attn pooled in bf16 anyway"):
                        for h in range(4):
                            hs = slice(h * 2 * NB * P, (h + 1) * 2 * NB * P)
                            pr = attp.tile([128, 2 * NB * P], BF16, tag="pr")
                            nc.vector.tensor_tensor(
                                out=pr[:].rearrange("m (k x) -> m k x", k=2),
                                in0=at_all[:, hs].rearrange(
                                    "m (k x) -> m k x", k=2),
                                in1=bass.AP(wfull[:].tensor, wfull[:].offset,
                                            [wfull[:].ap[0], [0, 2],
                                             wfull[:].ap[1]]),
                                op=ALU.mult)
                            ab4 = abtp.tile([128, 2 * NB], BF16, tag="ab")
                            nc.vector.tensor_reduce(
                                ab4[:],
                                pr[:].rearrange("m (k n p) -> m k n p", k=2,
                                                p=P),
                                axis=AXX, op=ALU.add)
                            for kk in range(2):
                                attnT.append(
                                    ab4[:, kk * NB:(kk + 1) * NB])
                    nc.leave_named_scope(f"att{t}", sc_[0], False)

                    # quad-stacked state for this step: blocks 4g'..4g'+3 of
                    # quad q live on partitions 32g'..32g'+31
                    GS = [blkp.tile([128, 512], F32, tag=f"GS{q}",
                                    name=f"GS{q}_{t}") for q in range(2)]
                    c_new = [blkp.tile([128, 128], F32, tag="c", bufs=4,
                                       name=f"cn{q}_{t}") for q in range(2)]
                    hbf = [blkp.tile([128, 128], BF16, tag="hbf",
                                     name=f"hbf{q}_{t}") for q in range(2)]
                    hT_new = []

                    def merge_block(g, pg):
                        # strip1 + strip0(+U) -> stacked row of GS[quad]
                        q, gp = divmod(g, 4)
                        row = slice(32 * gp, 32 * gp + 32)
                        g1 = blkp.tile([NB, 512], F32, tag="g1")
                        nc.scalar.activation(g1[:], pg[64:64 + NB, :], AF.Copy)
                        nc.vector.tensor_add(GS[q][row, :], pg[0:NB, :],
                                             g1[:])

                    def quad_math(q):
                        # all four blocks of the quad at full partition width
                        gq = GS[q]
                        sio = blkp.tile([128, 384], F32, tag="sio")
                        nc.scalar.activation(sio[:], gq[:, 0:384], AF.Sigmoid)
                        tg = blkp.tile([128, 128], F32, tag="tg")
                        nc.scalar.activation(tg[:], gq[:, 384:512], AF.Tanh)
                        m1 = blkp.tile([128, 128], F32, tag="m1")
                        nc.vector.tensor_tensor(out=m1[:], in0=sio[:, 0:128],
                                                in1=tg[:], op=ALU.mult)
                        m2 = blkp.tile([128, 128], F32, tag="m2")
                        nc.vector.tensor_tensor(out=m2[:], in0=sio[:, 128:256],
                                                in1=c_b[q][:], op=ALU.mult)
                        nc.vector.tensor_add(c_new[q][:], m1[:], m2[:])
                        tcn = blkp.tile([128, 128], F32, tag="tcn")
                        nc.scalar.activation(tcn[:], c_new[q][:], AF.Tanh)
                        nc.vector.tensor_tensor(out=hbf[q][:],
                                                in0=sio[:, 256:384],
                                                in1=tcn[:], op=ALU.mult)
                        if not last:
                            for gp in range(4):
                                ht_ = htp.tile([128, NB], BF16, tag="ht")
                                nc.sync.dma_start(
                                    ht_[:], hbf[q][32 * gp:32 * gp + 32, :],
                                    transpose=True)
                                hT_new.append(ht_)
                        # fp32 copy for the DRAM output
                        hf = blkp.tile([128, 128], F32, tag="hf",
                                       name=f"hf{q}_{t}")
                        nc.scalar.activation(hf[:], hbf[q][:], AF.Copy)
                        qsl = slice(q * 512, (q + 1) * 512)
                        nc.sync.dma_start(
                            out_d[:, t, qsl].rearrange("n (g c) -> g n c",
                                                       g=4),
                            hf[:])

                    # (f) attn-parts for blocks 0..5: first chunks k-outer so
                    # the matmuls pipeline against attnT production, then
                    # per-block finish; quad0 math after block 3's merge
                    sc_ = nc.enter_named_scope(f"f05_{t}", False)
                    for k in range(KH, KH + 4):
                        for g in range(6):
                            gsl = slice(g * 512, (g + 1) * 512)
                            smm(psg[g], k, attnT[k - KH], w2[k][:, gsl],
                                False, False)
                    for g in range(6):
                        gsl = slice(g * 512, (g + 1) * 512)
                        for k in range(KH + 4, K2):
                            smm(psg[g], k, attnT[k - KH], w2[k][:, gsl],
                                False, k >= K2 - 2)
                        merge_block(g, psg[g])
                        if g == 3:
                            quad_math(0)
                    nc.leave_named_scope(f"f05_{t}", sc_[0], False)

                    # (h) blocks 6,7: full accumulation + merges + quad1
                    sc_ = nc.enter_named_scope(f"h67_{t}", False)
                    for g in (6, 7):
                        gsl = slice(g * 512, (g + 1) * 512)
                        pg = psg_p.tile([128, 512], F32, tag="g",
                                        name=f"pg67_{g}")
                        for k in range(KH):
                            smm(pg, k, hT[k], w2[k][:, gsl], k < 2, False)
                        umm(pg, gsl, u_t)
                        for k in range(KH, K2):
                            smm(pg, k, attnT[k - KH], w2[k][:, gsl],
                                False, k >= K2 - 2)
                        merge_block(g, pg)
                    quad_math(1)
                    nc.leave_named_scope(f"h67_{t}", sc_[0], False)

                    # (i..l) next step's scores + blocks 0..3 h-part + U;
                    # hT chunks 0..3 arrive with quad0, 4..7 with quad1
                    sc_ = nc.enter_named_scope(f"nxt{t}", False)
                    if not last:
                        ps_s = pss_p.tile([NB, NB * P], F32, tag="s")
                        for k in range(4):
                            nc.tensor.matmul(ps_s[:], hT_new[k][:], at[k],
                                             start=(k == 0), stop=False)
                        psg2 = {}
                        for g in range(4):
                            gsl = slice(g * 512, (g + 1) * 512)
                            pg = psg_p.tile([128, 512], F32, tag="g",
                                            name=f"pgn{g}")
                            for k in range(4):
                                smm(pg, k, hT_new[k], w2[k][:, gsl],
                                    k < 2, False)
                            umm(pg, gsl, u_next)
                            psg2[g] = pg
                        for k in (4, 5, 6, 7):
                            nc.tensor.matmul(ps_s[:], hT_new[k][:], at[k],
                                             start=False, stop=(k == 7))
                        for g in range(4):
                            gsl = slice(g * 512, (g + 1) * 512)
                            for k in (4, 5, 6, 7):
                                smm(psg2[g], k, hT_new[k], w2[k][:, gsl],
                                    False, False)
                        psg = psg2
                    nc.leave_named_scope(f"nxt{t}", sc_[0], False)

                    if not last:
                        hT = hT_new
                        c_b = c_new
                        u_t = u_next

    nc.compile()
    return nc


def prepare_inputs(x, A, Wx, Wh, Wattn, b, t_steps=T):
    """Host-side sharding + layout prep. Returns list of per-core input maps."""
    x = np.asarray(x, dtype=np.float32)
    A = np.asarray(A, dtype=np.float32)
    Wx = np.asarray(Wx, dtype=np.float32)
    Wh = np.asarray(Wh, dtype=np.float32)
    Wattn = np.asarray(Wattn, dtype=np.float32)
    b = np.asarray(b, dtype=np.float32)

    perm = _gate_perm()
    wx_p = np.ascontiguousarray(Wx[:, perm]).astype(BF)
    w2_p = np.ascontiguousarray(np.vstack([Wh, Wattn])[:, perm]).astype(BF)
    b128 = np.ascontiguousarray(
        np.broadcast_to(b[perm], (128, G))).astype(np.float32)
    mask = np.zeros((NB, NB * P), dtype=BF)
    for n in range(NB):
        mask[n, n * P:(n + 1) * P] = 1
    ones = np.ones((1, 128), dtype=BF)
    ident = np.eye(NB, dtype=BF)

    in_maps = []
    for c in range(NCORES):
        x_c = x[c * NB:(c + 1) * NB, :t_steps]          # (NB, t, D)
        xr = x_c.transpose(1, 0, 2).reshape(t_steps * NB, D)  # t-major rows
        xT = np.ascontiguousarray(xr.T).astype(BF)       # (D, t*NB)
        A_c = A[c * NB:(c + 1) * NB].reshape(NB, H, P)
        at_c = np.ascontiguousarray(
            A_c.transpose(1, 0, 2).reshape(H, NB * P)).astype(BF)
        h0 = A_c.mean(axis=2).astype(np.float32)         # (NB, H)
        h0T = np.ascontiguousarray(h0.T).astype(BF)      # (H, NB)
        # quad-stacked initial cell state: block g of quad q lives on
        # partitions 32*(g%4), columns = h dims within the block
        h0q = np.empty((2 * 128, 128), dtype=np.float32)
        for g in range(8):
            q, gp = divmod(g, 4)
            h0q[q * 128 + gp * 32:q * 128 + (gp + 1) * 32, :] = \
                h0[:, g * 128:(g + 1) * 128]
        in_maps.append({
            "xT": xT, "wx": wx_p, "w2": w2_p, "b128": b128,
            "at": at_c, "h0T": h0T, "h0q": h0q,
            "mask": mask, "ones": ones, "ident": ident,
        })
    return in_maps


def kernel(x, A, Wx, Wh, Wattn, b):
    from concourse.bass_utils import run_bass_kernel_spmd

    key = T
    if key not in _NC_CACHE:
        _NC_CACHE[key] = build_nc(T)
    nc = _NC_CACHE[key]

    in_maps = prepare_inputs(x, A, Wx, Wh, Wattn, b)
    trace = bool(int(os.environ.get("KERNEL_TRACE", "0")))
    res = run_bass_kernel_spmd(nc, in_maps, core_ids=list(range(NCORES)),
                               trace=trace)
    if res.exec_time_ns is not None:
        print(f"HW exec time: {res.exec_time_ns} ns")
        kernel.last_exec_time_ns = res.exec_time_ns
    out = np.concatenate([r["out"] for r in res.results], axis=0)
    return out.astype(np.float32)


kernel.last_exec_time_ns = None



# revision 8
# speedup vs baseline: 1.7257x; 1.7257x over previous
"""Trainium2 Bass kernel for an attention-augmented LSTM (CaptioningRNN).

Reference computation (per batch n, T timesteps):
    A_flat = A.reshape(N, H, 16); h0 = c0 = A_flat.mean(-1)
    scores_t = (h_{t-1} @ A_flat) / sqrt(H); w = softmax(scores)
    attn_t = A_flat @ w
    a = x_t @ Wx + h_{t-1} @ Wh + attn_t @ Wattn + b
    i, f, o, g = split(a, 4); c_t = sig(f)*c + sig(i)*tanh(g); h_t = sig(o)*tanh(c_t)

Strategy: data-parallel over batch across 8 cores (32 batch rows each).
Per core:
  Phase A: U = x @ Wx + b precomputed for all timesteps (bf16 weights,
           rows t-major, 2-way PE column tiling to hide LDWEIGHTS) and
           staged to DRAM in bf16.
  Phase B: 64 recurrent steps. The gate matmul contracts [h; attn]
           (2048-dim, bf16) against W2 = [Wh; Wattn] with gate-interleaved
           columns. Four PE column tiles (tile_position=(0,32j)) run
           CONCURRENTLY, one gate block per 32-partition strip, so the
           128x512 PSUM tile is directly the quad-stacked (4 blocks x 32
           batch) gate layout used by the fused LSTM math - no merge ops.
           U enters each strip via an identity-matmul. Attention scores are
           all-pairs matmuls (also 4-way column-tiled, k-chunks striped
           across strips), extracted with a full-128-partition mask+reduce
           plus 3 tiny strip adds. h is transposed back to [h-dim, batch]
           with a PE-mode transpose (not DMA).

Weight-matrix column order (gate interleave): block j (512 cols) holds
original columns [i|f|o|g][j*128:(j+1)*128]. The same permutation is applied
to Wx, b and hence U.
"""

import math
import os

import numpy as np
import ml_dtypes

import concourse.bass as bass
import concourse.mybir as mybir
import concourse.tile as tile
from concourse import bacc

N, T, D, H = 256, 64, 1024, 1024
NCORES = 8
NB = N // NCORES          # 32 batch rows per core
G = 4 * H                 # 4096 gate columns
P = 16                    # attention positions (4x4)
KH = H // 128             # 8 contraction chunks for h
K2 = (2 * H) // 128       # 16 contraction chunks for [h; attn]
GB = G // 512             # 8 gate blocks of 512
F32 = mybir.dt.float32
BF16 = mybir.dt.bfloat16
BF = ml_dtypes.bfloat16

AF = mybir.ActivationFunctionType
ALU = mybir.AluOpType
AXX = mybir.AxisListType.X

_NC_CACHE = {}


def _gate_perm():
    """perm[new_col] = old_col for the gate-interleaved layout."""
    perm = np.empty(G, dtype=np.int64)
    for j in range(GB):
        for s in range(4):  # i, f, o, g
            perm[j * 512 + s * 128:(j * 512 + (s + 1) * 128)] = np.arange(
                s * H + j * 128, s * H + (j + 1) * 128)
    return perm


def build_nc(t_steps=T):
    """Build the SPMD Bass program (identical on all cores)."""
    nc = bacc.Bacc("TRN2", target_bir_lowering=False, debug=False,
                   num_devices=NCORES)

    xT_d = nc.dram_tensor("xT", [D, t_steps * NB], BF16, kind="ExternalInput")
    wx_d = nc.dram_tensor("wx", [D, G], BF16, kind="ExternalInput")
    w2_d = nc.dram_tensor("w2", [2 * H, G], BF16, kind="ExternalInput")
    b128_d = nc.dram_tensor("b128", [128, G], F32, kind="ExternalInput")
    at_d = nc.dram_tensor("at", [H, NB * P], BF16, kind="ExternalInput")
    h0t2_d = nc.dram_tensor("h0t2", [2 * 128, 128], BF16, kind="ExternalInput")
    h0q_d = nc.dram_tensor("h0q", [2 * 128, 128], F32, kind="ExternalInput")
    mask_d = nc.dram_tensor("mask", [128, NB * P], BF16, kind="ExternalInput")
    ones_d = nc.dram_tensor("ones", [1, 128], BF16, kind="ExternalInput")
    ident_d = nc.dram_tensor("ident", [NB, NB], BF16, kind="ExternalInput")
    id128_d = nc.dram_tensor("id128", [128, 128], BF16, kind="ExternalInput")
    sum4_d = nc.dram_tensor("sum4", [128, NB], F32, kind="ExternalInput")
    out_d = nc.dram_tensor("out", [NB, t_steps, H], BF16,
                           kind="ExternalOutput")

    n_row_tiles = (t_steps * NB) // 128

    with tile.TileContext(nc) as tc:
        with tc.tile_pool(name="dram", bufs=1, space="DRAM") as dpool:
            u_dram = dpool.tile([t_steps * NB, G], BF16)

            # ---------------- Phase A: U = x @ Wx + b ----------------
            # 2-way PE column tiling (M=64 halves) so each half's
            # LDWEIGHTS hides under the other half's 512-col stream.
            with tc.tile_pool(name="pa_res", bufs=1) as pa, \
                 tc.tile_pool(name="pa_ps", bufs=4, space="PSUM") as pa_ps, \
                 tc.tile_pool(name="pa_sb", bufs=6) as pa_sb:
                xT = []
                for d in range(KH):
                    t_ = pa.tile([128, t_steps * NB], BF16, tag=f"xT{d}")
                    nc.sync.dma_start(t_[:], xT_d[d * 128:(d + 1) * 128, :])
                    xT.append(t_)
                wx = []
                for d in range(KH):
                    t_ = pa.tile([128, G], BF16, tag=f"wx{d}")
                    nc.sync.dma_start(t_[:], wx_d[d * 128:(d + 1) * 128, :])
                    wx.append(t_)
                b128 = pa.tile([128, G], F32, tag="b128")
                nc.sync.dma_start(b128[:], b128_d[:])

                for m in range(n_row_tiles):
                    for g in range(GB):
                        gs = slice(g * 512, (g + 1) * 512)
                        ps = pa_ps.tile([128, 512], F32, tag="ps")
                        for d in range(KH):
                            for j in range(2):
                                ms = slice(m * 128 + 64 * j,
                                           m * 128 + 64 * j + 64)
                                nc.tensor.matmul(
                                    ps[64 * j:64 * j + 64, :],
                                    xT[d][:, ms], wx[d][:, gs],
                                    start=(d == 0), stop=(d == KH - 1),
                                    tile_position=(0, 64 * j),
                                    skip_group_check=True)
                        us = pa_sb.tile([128, 512], BF16, tag="us")
                        nc.vector.tensor_add(us[:], ps[:], b128[:, gs])
                        ms_full = slice(m * 128, (m + 1) * 128)
                        nc.sync.dma_start(u_dram[ms_full, gs], us[:])

            # ---------------- Phase B: recurrence ----------------
            with tc.tile_pool(name="res", bufs=1) as res, \
                 tc.tile_pool(name="ht", bufs=3) as htp, \
                 tc.tile_pool(name="u", bufs=2) as up, \
                 tc.tile_pool(name="st", bufs=2) as stp, \
                 tc.tile_pool(name="att", bufs=2) as attp, \
                 tc.tile_pool(name="abt", bufs=10) as abtp, \
                 tc.tile_pool(name="blk", bufs=2) as blkp, \
                 tc.tile_pool(name="psg", bufs=4, space="PSUM") as psg_p, \
                 tc.tile_pool(name="pss", bufs=1, space="PSUM") as pss_p, \
                 tc.tile_pool(name="psw", bufs=1, space="PSUM") as psw_p, \
                 tc.tile_pool(name="pst", bufs=1, space="PSUM") as pst_p, \
                 tc.tile_pool(name="psc", bufs=1, space="PSUM") as psc_p:

                w2 = []
                for k in range(K2):
                    t_ = res.tile([128, G], BF16, tag=f"w2_{k}")
                    nc.sync.dma_start(t_[:], w2_d[k * 128:(k + 1) * 128, :])
                    w2.append(t_)
                at_all = res.tile([128, KH * NB * P], BF16, tag="at_all")
                for k in range(KH):
                    nc.sync.dma_start(
                        at_all[:, k * NB * P:(k + 1) * NB * P],
                        at_d[k * 128:(k + 1) * 128, :])
                at = [at_all[:, k * NB * P:(k + 1) * NB * P]
                      for k in range(KH)]
                mask = res.tile([128, NB * P], BF16, tag="mask")
                nc.sync.dma_start(mask[:], mask_d[:])
                ones = res.tile([1, 128], BF16, tag="ones")
                nc.sync.dma_start(ones[:], ones_d[:])
                ident = res.tile([NB, NB], BF16, tag="ident")
                nc.sync.dma_start(ident[:], ident_d[:])
                id128 = res.tile([128, 128], BF16, tag="id128")
                nc.sync.dma_start(id128[:], id128_d[:])
                sum4 = res.tile([128, NB], F32, tag="sum4")
                nc.sync.dma_start(sum4[:], sum4_d[:])

                # hT as two packed [128, 128] tiles: tile q column 32*g+n
                # holds h[n, 128*(4q+g) + c] for partition c.
                hTq = []
                for q in range(2):
                    t_ = htp.tile([128, 128], BF16, tag="htq",
                                  name=f"h0t{q}")
                    nc.sync.dma_start(t_[:], h0t2_d[q * 128:(q + 1) * 128, :])
                    hTq.append(t_)

                def ht_chunk(k):
                    q, gp = divmod(k, 4)
                    return hTq[q][:, 32 * gp:32 * gp + 32]

                c_b = []
                for q in range(2):
                    t_ = blkp.tile([128, 128], F32, tag="c", bufs=4,
                                   name=f"c0_{q}")
                    nc.sync.dma_start(t_[:], h0q_d[q * 128:(q + 1) * 128, :])
                    c_b.append(t_)

                u_t = up.tile([NB, G], BF16, tag="u")
                nc.sync.dma_start(u_t[:], u_dram[0:NB, :])

                inv_sqrt_h = 1.0 / math.sqrt(H)

                def gmm(pg, j, lhs, rhs, start, stop):
                    """One gate matmul into strip j (col tile (0, 32j))."""
                    nc.tensor.matmul(pg[32 * j:32 * j + NB, :], lhs, rhs,
                                     start=start, stop=stop,
                                     tile_position=(0, 32 * j),
                                     skip_group_check=True)

                def umm(pg, j, u, gsl):
                    """Init strip j of pg with the U slice (ident matmul)."""
                    nc.tensor.matmul(pg[32 * j:32 * j + NB, :], ident[:],
                                     u[:, gsl], start=True, stop=False,
                                     tile_position=(0, 32 * j),
                                     skip_group_check=True)

                def smm(ps_s, j, k, hts, start):
                    """Score partial for k-chunk k into strip j."""
                    nc.tensor.matmul(ps_s[32 * j:32 * j + NB, :], hts, at[k],
                                     start=start, stop=False,
                                     tile_position=(0, 32 * j),
                                     skip_group_check=True)

                def new_gate_psums(t):
                    return [psg_p.tile([128, 512], F32, tag="g",
                                       name=f"pg{q}_{t}") for q in range(2)]

                def issue_umm(psg, u):
                    for q in range(2):
                        for j in range(4):
                            g = 4 * q + j
                            umm(psg[q], j, u, slice(g * 512, (g + 1) * 512))

                def issue_h_part(psg, hts):
                    # rounds: all 4 strips of a quad run concurrently
                    for k in range(KH):
                        for q in range(2):
                            for j in range(4):
                                g = 4 * q + j
                                gsl = slice(g * 512, (g + 1) * 512)
                                gmm(psg[q], j, hts(k), w2[k][:, gsl],
                                    False, False)

                def issue_scores(hts, t):
                    ps_s = pss_p.tile([128, NB * P], F32, tag="s",
                                      name=f"s{t}")
                    for k in range(KH):
                        smm(ps_s, k % 4, k, hts(k), start=(k < 4))
                    return ps_s

                # ---- prologue: scores S_0, U_0 + h-part of all strips ----
                ps_s = issue_scores(ht_chunk, 0)
                psg = new_gate_psums(0)
                issue_umm(psg, u_t)
                issue_h_part(psg, ht_chunk)

                for t in range(t_steps):
                    last = (t + 1 >= t_steps)
                    if not last:
                        u_next = up.tile([NB, G], BF16, tag="u")
                        nc.scalar.dma_start(
                            u_next[:], u_dram[(t + 1) * NB:(t + 2) * NB, :])

                    # (a) score extraction + softmax chain
                    sm_sc = nc.enter_named_scope(f"sm{t}", False)
                    masked = stp.tile([128, NB * P], F32, tag="masked")
                    nc.vector.tensor_tensor(
                        out=masked[:].rearrange("m (p n) -> m p n", n=NB),
                        in0=ps_s[:].rearrange("m (n p) -> m p n", p=P),
                        in1=mask[:].rearrange("m (n p) -> m p n", p=P),
                        op=ALU.mult)
                    sc4 = stp.tile([128, P], F32, tag="sc4")
                    nc.vector.tensor_reduce(
                        sc4[:], masked[:].rearrange("m (p n) -> m p n", n=NB),
                        axis=AXX, op=ALU.add)
                    # sum the 4 k-striped partials on the PE: stacked-identity
                    # contraction (K=128 -> M=32), fp32 matmul, tiny N=16
                    sc = psc_p.tile([NB, P], F32, tag="scps")
                    nc.tensor.matmul(sc[:], sum4[:], sc4[:],
                                     start=True, stop=True)
                    # exp(x) = s/(1-s) with s = sigmoid(x): keeps the ACT
                    # table cache at {Sigmoid, Tanh} with no per-step reloads
                    sg = stp.tile([NB, P], F32, tag="sg")
                    nc.scalar.activation(sg[:], sc[:], AF.Sigmoid,
                                         scale=float(inv_sqrt_h))
                    om = stp.tile([NB, P], F32, tag="om")
                    nc.scalar.activation(om[:], sc[:], AF.Sigmoid,
                                         scale=float(-inv_sqrt_h))
                    omr = stp.tile([NB, P], F32, tag="omr")
                    nc.vector.reciprocal(omr[:], om[:])
                    expw = stp.tile([NB, P], F32, tag="expw")
                    nc.vector.tensor_tensor(out=expw[:], in0=sg[:], in1=omr[:],
                                            op=ALU.mult)
                    sume = stp.tile([NB, 1], F32, tag="sume")
                    nc.vector.tensor_reduce(sume[:], expw[:], axis=AXX,
                                            op=ALU.add)
                    rec = stp.tile([NB, 1], F32, tag="rec")
                    nc.vector.reciprocal(rec[:], sume[:])
                    w16 = stp.tile([NB, P], BF16, tag="w16")
                    nc.vector.tensor_scalar(out=w16[:], in0=expw[:],
                                            scalar1=rec[:], scalar2=None,
                                            op0=ALU.mult)
                    # flatten [NB, P] -> [1, NB*P]: direct SBUF->SBUF gather
                    w1 = stp.tile([1, NB * P], BF16, tag="w1")
                    nc.scalar.dma_start(w1[:], w16[:])
                    nc.leave_named_scope(f"sm{t}", sm_sc[0], False)

                    # (b) broadcast w to 128 partitions via ones-matmul
                    sc_ = nc.enter_named_scope(f"att{t}", False)
                    ps_w = psw_p.tile([128, NB * P], F32, tag="w")
                    nc.tensor.matmul(ps_w[:], ones[:], w1[:],
                                     start=True, stop=True)
                    wfull = attp.tile([128, NB * P], BF16, tag="wfull")
                    nc.scalar.activation(wfull[:], ps_w[:], AF.Copy)

                    # (c) attention pooling -> attnT: two wide DVE
                    # product+reduce pairs over the packed AT tile
                    attnT = []
                    with nc.allow_low_precision("attn pooled in bf16 anyway"):
                        for h in range(4):
                            hs = slice(h * 2 * NB * P, (h + 1) * 2 * NB * P)
                            pr = attp.tile([128, 2 * NB * P], BF16, tag="pr")
                            nc.vector.tensor_tensor(
                                out=pr[:].rearrange("m (k x) -> m k x", k=2),
                                in0=at_all[:, hs].rearrange(
                                    "m (k x) -> m k x", k=2),
                                in1=bass.AP(wfull[:].tensor, wfull[:].offset,
                                            [wfull[:].ap[0], [0, 2],
                                             wfull[:].ap[1]]),
                                op=ALU.mult)
                            ab4 = abtp.tile([128, 2 * NB], BF16, tag="ab")
                            nc.vector.tensor_reduce(
                                ab4[:],
                                pr[:].rearrange("m (k n p) -> m k n p", k=2,
                                                p=P),
                                axis=AXX, op=ALU.add)
                            for kk in range(2):
                                attnT.append(
                                    ab4[:, kk * NB:(kk + 1) * NB])
                    nc.leave_named_scope(f"att{t}", sc_[0], False)

                    # (d) attn-part matmuls: 4 strips per quad concurrent,
                    # rounds follow attnT chunk production order
                    sc2 = nc.enter_named_scope(f"ga{t}", False)
                    for k in range(KH, K2):
                        for q in range(2):
                            for j in range(4):
                                g = 4 * q + j
                                gsl = slice(g * 512, (g + 1) * 512)
                                gmm(psg[q], j, attnT[k - KH], w2[k][:, gsl],
                                    False, k == K2 - 1)
                    nc.leave_named_scope(f"ga{t}", sc2[0], False)

                    # (e) fused LSTM math on the quad-stacked psum tiles
                    sc3 = nc.enter_named_scope(f"qm{t}", False)
                    c_new = [blkp.tile([128, 128], F32, tag="c", bufs=4,
                                       name=f"cn{q}_{t}") for q in range(2)]
                    hbf = [blkp.tile([128, 128], BF16, tag="hbf",
                                     name=f"hbf{q}_{t}") for q in range(2)]

                    for q in range(2):
                        gq = psg[q]
                        sio = blkp.tile([128, 384], F32, tag="sio")
                        nc.scalar.activation(sio[:], gq[:, 0:384], AF.Sigmoid)
                        tg = blkp.tile([128, 128], F32, tag="tg")
                        nc.scalar.activation(tg[:], gq[:, 384:512], AF.Tanh)
                        m1 = blkp.tile([128, 128], F32, tag="m1")
                        nc.vector.tensor_tensor(out=m1[:], in0=sio[:, 0:128],
                                                in1=tg[:], op=ALU.mult)
                        m2 = blkp.tile([128, 128], F32, tag="m2")
                        nc.vector.tensor_tensor(out=m2[:], in0=sio[:, 128:256],
                                                in1=c_b[q][:], op=ALU.mult)
                        nc.vector.tensor_add(c_new[q][:], m1[:], m2[:])
                        tcn = blkp.tile([128, 128], F32, tag="tcn")
                        nc.scalar.activation(tcn[:], c_new[q][:], AF.Tanh)
                        nc.vector.tensor_tensor(out=hbf[q][:],
                                                in0=sio[:, 256:384],
                                                in1=tcn[:], op=ALU.mult)
                        qsl = slice(q * 512, (q + 1) * 512)
                        nc.sync.dma_start(
                            out_d[:, t, qsl].rearrange("n (g c) -> g n c",
                                                       g=4),
                            hbf[q][:])
                    nc.leave_named_scope(f"qm{t}", sc3[0], False)

                    # (f) next step: U-init during the transpose window,
                    # then PE transpose h -> hT, scores, h-part matmuls
                    sc4_ = nc.enter_named_scope(f"nxt{t}", False)
                    if not last:
                        psg2 = new_gate_psums(t + 1)
                        issue_umm(psg2, u_next)
                        tr_ps = pst_p.tile([128, 256], BF16, tag="tr",
                                           name=f"tr{t}")
                        hT_new = []
                        for q in range(2):
                            nc.tensor.transpose(
                                tr_ps[:, q * 128:(q + 1) * 128],
                                hbf[q][:], id128[:])
                            ht_ = htp.tile([128, 128], BF16, tag="htq",
                                           name=f"ht{q}_{t}")
                            nc.scalar.activation(ht_[:],
                                                 tr_ps[:, q * 128:
                                                       (q + 1) * 128],
                                                 AF.Copy)
                            hT_new.append(ht_)

                        def hts_new(k):
                            q, gp = divmod(k, 4)
                            return hT_new[q][:, 32 * gp:32 * gp + 32]

                        ps_s = issue_scores(hts_new, t + 1)
                        issue_h_part(psg2, hts_new)
                        psg = psg2
                        hTq = hT_new
                        c_b = c_new
                        u_t = u_next
                    nc.leave_named_scope(f"nxt{t}", sc4_[0], False)

    nc.compile()
    return nc


def prepare_inputs(x, A, Wx, Wh, Wattn, b, t_steps=T):
    """Host-side sharding + layout prep. Returns list of per-core input maps."""
    x = np.asarray(x, dtype=np.float32)
    A = np.asarray(A, dtype=np.float32)
    Wx = np.asarray(Wx, dtype=np.float32)
    Wh = np.asarray(Wh, dtype=np.float32)
    Wattn = np.asarray(Wattn, dtype=np.float32)
    b = np.asarray(b, dtype=np.float32)

    perm = _gate_perm()
    wx_p = np.ascontiguousarray(Wx[:, perm]).astype(BF)
    w2_p = np.ascontiguousarray(np.vstack([Wh, Wattn])[:, perm]).astype(BF)
    b128 = np.ascontiguousarray(
        np.broadcast_to(b[perm], (128, G))).astype(np.float32)
    mask32 = np.zeros((NB, NB * P), dtype=BF)
    for n in range(NB):
        mask32[n, n * P:(n + 1) * P] = 1
    mask = np.ascontiguousarray(np.tile(mask32, (4, 1)))
    ones = np.ones((1, 128), dtype=BF)
    ident = np.eye(NB, dtype=BF)
    id128 = np.eye(128, dtype=BF)
    sum4 = np.ascontiguousarray(
        np.tile(np.eye(NB, dtype=np.float32), (4, 1)))

    in_maps = []
    for c in range(NCORES):
        x_c = x[c * NB:(c + 1) * NB, :t_steps]          # (NB, t, D)
        xr = x_c.transpose(1, 0, 2).reshape(t_steps * NB, D)  # t-major rows
        xT = np.ascontiguousarray(xr.T).astype(BF)       # (D, t*NB)
        A_c = A[c * NB:(c + 1) * NB].reshape(NB, H, P)
        at_c = np.ascontiguousarray(
            A_c.transpose(1, 0, 2).reshape(H, NB * P)).astype(BF)
        h0 = A_c.mean(axis=2).astype(np.float32)         # (NB, H)
        # quad-stacked initial cell state: block g of quad q lives on
        # partitions 32*(g%4), columns = h dims within the block
        h0q = np.empty((2 * 128, 128), dtype=np.float32)
        for g in range(8):
            q, gp = divmod(g, 4)
            h0q[q * 128 + gp * 32:q * 128 + (gp + 1) * 32, :] = \
                h0[:, g * 128:(g + 1) * 128]
        # packed transposed h0: tile q, col 32*gp+n, row c
        h0t2 = np.ascontiguousarray(
            h0q.reshape(2, 128, 128).transpose(0, 2, 1)
            .reshape(2 * 128, 128)).astype(BF)
        in_maps.append({
            "xT": xT, "wx": wx_p, "w2": w2_p, "b128": b128,
            "at": at_c, "h0t2": h0t2, "h0q": h0q,
            "mask": mask, "ones": ones, "ident": ident, "id128": id128,
            "sum4": sum4,
        })
    return in_maps


def kernel(x, A, Wx, Wh, Wattn, b):
    from concourse.bass_utils import run_bass_kernel_spmd

    key = T
    if key not in _NC_CACHE:
        _NC_CACHE[key] = build_nc(T)
    nc = _NC_CACHE[key]

    in_maps = prepare_inputs(x, A, Wx, Wh, Wattn, b)
    trace = bool(int(os.environ.get("KERNEL_TRACE", "0")))
    res = run_bass_kernel_spmd(nc, in_maps, core_ids=list(range(NCORES)),
                               trace=trace)
    if res.exec_time_ns is not None:
        print(f"HW exec time: {res.exec_time_ns} ns")
        kernel.last_exec_time_ns = res.exec_time_ns
    out = np.concatenate([r["out"] for r in res.results], axis=0)
    return out.astype(np.float32)


kernel.last_exec_time_ns = None
